# revision 1
# baseline (speedup 1.0000x reference)
# Trainium2 Bass kernel for nn_GTEProgramClassification (GNN message passing).
#
# Math (see problem reference):
#   feat_src = mean_s emb[token_id[:, s]]              [N_src, D]
#   msgs     = feat_src[neigh_idx]                     [N_dst, max_deg, D]
#   h        = GRU scan over msgs[:, :deg-1]           (per-node step count)
#   ft       = deg==1 ? msgs[:,0] : LayerNorm(h)
#   out      = ft @ wc.T + bc                          [N_dst, n_classes]
#
# Strategy (8 cores, data-parallel over dst nodes):
#   * Host sorts dst nodes by degree (descending) and deals them across the
#     8 cores so every core sees the exact same degree profile (classes are
#     padded to a multiple of 8 with fake nodes). One SPMD program.
#   * At GRU step t the active nodes are a shrinking prefix of the sorted
#     columns; retired columns keep their final h in place.
#   * feat_src is never materialized: per (step, 512-node chunk) we
#     dma_gather the 4 sub-token embedding rows of every active node
#     (int16 indices, interleaved so one node's 4 rows land 32 partitions
#     apart), then a single PE matmul per 32 nodes against a constant
#     [128,32] selection matrix R32 (R32[p,n] = 0.25*(p%32==n)) performs
#     sum-over-subtokens + transpose + mean scaling in one op.
#   * Layout B on chip: tiles are [D=128 partitions, nodes free].  LayerNorm
#     reductions over D are K=128 matmuls with a 1/128 ones vector;
#     per-node scalars are partition-broadcast via K=1 matmuls.
#   * All large matmuls run in float32r (1 cycle/row vs 4 for fp32; HW
#     rel err ~1e-4): producers (DVE/ACT) round into fp32r tiles.
#   * Classifier emits [n_classes, nodes]; host transposes / un-permutes.

import numpy as np

import concourse.bacc as bacc
import concourse.bass as bass
import concourse.mybir as mybir
import concourse.tile as tile
from concourse import bass_utils

N_SRC = 50000
N_DST = 32000
MAX_DEG = 16
N_SUB = 4
D = 128
VOCAB = 32000
N_CLASSES = 104
LN_EPS = 1e-5
T = MAX_DEG - 1
NCORES = 8
P = 128
CH = 512  # free-dim chunk (one PSUM bank of f32)

F32 = mybir.dt.float32
R32DT = mybir.dt.float32r
I16 = mybir.dt.int16
AF = mybir.ActivationFunctionType
ALU = mybir.AluOpType


def _r32up(n):
    return (n + 31) // 32 * 32


# ----------------------------------------------------------------- host prep

def _schedule(deg):
    """Shared per-core schedule from the degree histogram."""
    counts = np.bincount(deg, minlength=MAX_DEG + 1)
    # Pad each degree class to a multiple of 2*NCORES: every core gets the
    # same class counts AND every per-core count is even, so all chunk
    # widths are even (fp32r matmuls require an even moving free dim).
    pm = 2 * NCORES
    cnt_pad = ((counts + pm - 1) // pm) * pm
    cnt_core = cnt_pad // NCORES
    deg_col = np.concatenate(
        [np.full(cnt_core[d], d, np.int64) for d in range(MAX_DEG, 0, -1)]
    )
    Ncol = int(len(deg_col))
    n_t = [int((deg_col >= t + 2).sum()) for t in range(T)]
    N2 = n_t[0]
    Nd1 = Ncol - N2
    kd1 = (Nd1 + P - 1) // P

    # index-buffer offsets (int16 columns), one entry per (step|deg1, chunk)
    offs = {}
    off = 0
    def add(key, w):
        nonlocal off
        L = N_SUB * _r32up(w)
        offs[key] = (off, L)
        off += L // 16
    for t in range(T):
        for c0 in range(0, n_t[t], CH):
            add((t, c0), min(CH, n_t[t] - c0))
    for c0 in range(0, Nd1, CH):
        add(("d1", c0), min(CH, Nd1 - c0))
    return dict(cnt_pad=cnt_pad, Ncol=Ncol, N2=N2, Nd1=Nd1, n_t=n_t, kd1=kd1,
                offs=offs, F16=off)


def _prep(token_id, neigh_idx, deg):
    token_id = np.asarray(token_id).astype(np.int16)  # VOCAB=32000 < 2**15
    neigh_idx = np.asarray(neigh_idx).astype(np.int64)
    deg = np.asarray(deg).astype(np.int64)
    sched = _schedule(deg)
    cnt_pad = sched["cnt_pad"]

    colnode = [[] for _ in range(NCORES)]
    for d in range(MAX_DEG, 0, -1):
        ids = np.where(deg == d)[0].astype(np.int64)
        padded = np.concatenate([ids, np.full(cnt_pad[d] - len(ids), -1, np.int64)])
        for c in range(NCORES):
            colnode[c].append(padded[c::NCORES])
    colnode = [np.concatenate(x) for x in colnode]

    F16 = sched["F16"]
    idxbufs = []
    for c in range(NCORES):
        nid = colnode[c]
        buf = np.zeros((P, F16), np.int16)

        def pack(nida, t_col, w, off, L):
            w32 = L // N_SUB
            src = np.where(nida >= 0, neigh_idx[np.clip(nida, 0, None), t_col], 0)
            tok4 = np.zeros((w32, N_SUB), np.int16)
            tok4[:w] = token_id[src]                      # [w, 4]
            # i = blk*128 + s*32 + n  ->  tok4[blk*32+n, s]
            seq = tok4.reshape(w32 // 32, 32, N_SUB).transpose(0, 2, 1).reshape(-1)
            p16 = seq.reshape(L // 16, 16).T              # [16, L/16]
            buf[:, off:off + L // 16] = np.tile(p16, (8, 1))

        for (key, (off, L)) in sched["offs"].items():
            if key[0] == "d1":
                c0 = key[1]
                w = min(CH, sched["Nd1"] - c0)
                pack(nid[sched["N2"] + c0: sched["N2"] + c0 + w], 0, w, off, L)
            else:
                t, c0 = key
                w = min(CH, sched["n_t"][t] - c0)
                pack(nid[c0:c0 + w], t, w, off, L)
        idxbufs.append(buf)
    return sched, idxbufs, colnode


def _params(emb, w_ih, w_hh, b_ih, b_hh, gamma, beta, wc, bc):
    f = np.float32
    emb = np.ascontiguousarray(np.asarray(emb), dtype=f)
    w_ih = np.asarray(w_ih).astype(f)
    w_hh = np.asarray(w_hh).astype(f)
    b_ih = np.asarray(b_ih).astype(f)
    b_hh = np.asarray(b_hh).astype(f)
    r32 = np.zeros((P, 32), f)
    r32[np.arange(P), np.arange(P) % 32] = 1.0 / N_SUB
    return dict(
        emb=emb,
        r32=r32,
        w_ihT=np.ascontiguousarray(w_ih.T),          # [D, 3D]
        w_hhT=np.ascontiguousarray(w_hh.T),          # [D, 3D]
        brc=(b_ih[0:D] + b_hh[0:D]).reshape(D, 1).copy(),
        bzc=(b_ih[D:2 * D] + b_hh[D:2 * D]).reshape(D, 1).copy(),
        bihn=b_ih[2 * D:3 * D].reshape(D, 1).copy(),
        bhhn=b_hh[2 * D:3 * D].reshape(1, D).copy(),
        wcT=np.ascontiguousarray(np.asarray(wc).astype(f).T),   # [D, n_classes]
        bcc=np.asarray(bc).astype(f).reshape(N_CLASSES, 1).copy(),
        gamma=np.asarray(gamma).astype(f).reshape(D, 1).copy(),
        beta=np.asarray(beta).astype(f).reshape(D, 1).copy(),
    )


# ------------------------------------------------------------ device program

def _build_program(sched):
    n_t = sched["n_t"]
    N2, Nd1, kd1, Ncol = sched["N2"], sched["Nd1"], sched["kd1"], sched["Ncol"]
    offs, F16 = sched["offs"], sched["F16"]
    Hw = N2 + kd1 * P  # h tile free size (>= Ncol)

    nc = bacc.Bacc("TRN2", target_bir_lowering=False, debug=False,
                   enable_asserts=False)

    emb = nc.dram_tensor("emb", [VOCAB, D], F32, kind="ExternalInput").ap()
    idxbuf = nc.dram_tensor("idxbuf", [P, F16], I16, kind="ExternalInput").ap()
    r32_d = nc.dram_tensor("r32", [P, 32], F32, kind="ExternalInput").ap()
    w_ihT_d = nc.dram_tensor("w_ihT", [D, 3 * D], F32, kind="ExternalInput").ap()
    w_hhT_d = nc.dram_tensor("w_hhT", [D, 3 * D], F32, kind="ExternalInput").ap()
    brc_d = nc.dram_tensor("brc", [D, 1], F32, kind="ExternalInput").ap()
    bzc_d = nc.dram_tensor("bzc", [D, 1], F32, kind="ExternalInput").ap()
    bihn_d = nc.dram_tensor("bihn", [D, 1], F32, kind="ExternalInput").ap()
    bhhn_d = nc.dram_tensor("bhhn", [1, D], F32, kind="ExternalInput").ap()
    wcT_d = nc.dram_tensor("wcT", [D, N_CLASSES], F32, kind="ExternalInput").ap()
    bcc_d = nc.dram_tensor("bcc", [N_CLASSES, 1], F32, kind="ExternalInput").ap()
    gamma_d = nc.dram_tensor("gamma", [D, 1], F32, kind="ExternalInput").ap()
    beta_d = nc.dram_tensor("beta", [D, 1], F32, kind="ExternalInput").ap()
    outT = nc.dram_tensor("outT", [N_CLASSES, Ncol], F32, kind="ExternalOutput").ap()

    with tile.TileContext(nc) as tc, \
         tc.tile_pool(name="consts", bufs=1) as consts, \
         tc.tile_pool(name="spool", bufs=4) as spool, \
         tc.tile_pool(name="mpool", bufs=3) as mpool, \
         tc.tile_pool(name="gpool", bufs=2) as gpool, \
         tc.tile_pool(name="hpool", bufs=1) as hpool, \
         tc.tile_pool(name="opool", bufs=2) as opool, \
         tc.tile_pool(name="ppool", bufs=1, space="PSUM") as ppool, \
         tc.tile_pool(name="trpool", bufs=2, space="PSUM") as trpool:

        def load(name, dram, shape, dtype=F32):
            t = consts.tile(shape, dtype, name=name)
            nc.sync.dma_start(out=t[:], in_=dram)
            return t

        sb_idx = load("sb_idx", idxbuf, [P, F16], I16)
        sbR32 = load("sbR32", r32_d, [P, 32])
        w_ihT_f = load("w_ihT_f", w_ihT_d, [D, 3 * D])
        w_hhT_f = load("w_hhT_f", w_hhT_d, [D, 3 * D])
        brc = load("brc_sb", brc_d, [D, 1])
        bzc = load("bzc_sb", bzc_d, [D, 1])
        bihn = load("bihn_sb", bihn_d, [D, 1])
        bhhn_f = load("bhhn_f", bhhn_d, [1, D])
        wcT_f = load("wcT_f", wcT_d, [D, N_CLASSES])
        bcc = load("bcc_sb", bcc_d, [N_CLASSES, 1])
        gamma = load("gamma_sb", gamma_d, [D, 1])
        beta = load("beta_sb", beta_d, [D, 1])

        # fp32r copies of everything that feeds fp32r matmuls
        w_ihT = consts.tile([D, 3 * D], R32DT, name="w_ihT_r")
        nc.vector.tensor_copy(w_ihT[:], w_ihT_f[:])
        w_hhT = consts.tile([D, 3 * D], R32DT, name="w_hhT_r")
        nc.vector.tensor_copy(w_hhT[:], w_hhT_f[:])
        bhhn = consts.tile([1, D], R32DT, name="bhhn_r")
        nc.vector.tensor_copy(bhhn[:], bhhn_f[:])
        wcT = consts.tile([D, N_CLASSES], R32DT, name="wcT_r")
        nc.vector.tensor_copy(wcT[:], wcT_f[:])

        ones_f = consts.tile([1, CH], F32, name="ones_f")
        nc.vector.memset(ones_f[:], 1.0)
        ones_r = consts.tile([1, CH], R32DT, name="ones_r")
        nc.vector.tensor_copy(ones_r[:], ones_f[:])
        onePf = consts.tile([D, 1], F32, name="onePf")
        nc.vector.memset(onePf[:], 1.0 / D)
        oneP = consts.tile([D, 1], R32DT, name="oneP_r")
        nc.vector.tensor_copy(oneP[:], onePf[:])
        eps_row = consts.tile([1, 1], F32, name="eps_row")
        nc.vector.memset(eps_row[:], LN_EPS)

        h = hpool.tile([P, Hw], R32DT, name="h")

        def mean_msgT(key, w):
            """Gather + fused subtoken-sum/transpose/0.25 -> [P, w32] fp32r."""
            off, L = offs[key]
            w32 = L // N_SUB
            S4 = spool.tile([P, L], F32, tag="S4", name="S4",
                            padded_shape=[P, N_SUB * CH])
            nc.gpsimd.dma_gather(
                out_ap=S4[:].rearrange("p (j e) -> p j e", e=P),
                in_ap=emb, idxs_ap=sb_idx[:, off:off + L // 16],
                num_idxs=L, num_idxs_reg=L, elem_size=P, single_packet=False)
            psumT = trpool.tile([P, CH], F32, tag="tr", name="psumT")
            for q in range(w32 // 32):
                nc.tensor.matmul(psumT[:, q * 32:(q + 1) * 32],
                                 S4[:, q * P:(q + 1) * P], sbR32[:],
                                 start=True, stop=True)
            msgT = mpool.tile([P, CH], R32DT, tag="msgT", name="msgT")
            nc.scalar.copy(msgT[:, :w32], psumT[:, :w32])
            return msgT

        # ---------------- GRU sweep over steps (descending-degree prefix) --
        for t in range(T):
            nt = n_t[t]
            for c0 in range(0, nt, CH):
                w = min(CH, nt - c0)
                cc = slice(c0, c0 + w)
                msgT = mean_msgT((t, c0), w)

                pr = ppool.tile([P, CH], F32, tag="pa", name="pr")
                pz = ppool.tile([P, CH], F32, tag="pb", name="pz")
                pnh = ppool.tile([P, CH], F32, tag="pc", name="pnh")
                pnx = ppool.tile([P, CH], F32, tag="pd", name="pnx")

                nc.tensor.matmul(pr[:, :w], w_ihT[:, 0:D], msgT[:, :w],
                                 start=True, stop=(t == 0))
                nc.tensor.matmul(pz[:, :w], w_ihT[:, D:2 * D], msgT[:, :w],
                                 start=True, stop=(t == 0))
                nc.tensor.matmul(pnx[:, :w], w_ihT[:, 2 * D:3 * D], msgT[:, :w],
                                 start=True, stop=True)
                nc.tensor.matmul(pnh[:, :w], bhhn[:1, :], ones_r[:1, :w],
                                 start=True, stop=(t == 0))
                if t > 0:
                    nc.tensor.matmul(pr[:, :w], w_hhT[:, 0:D], h[:, cc],
                                     start=False, stop=True)
                    nc.tensor.matmul(pz[:, :w], w_hhT[:, D:2 * D], h[:, cc],
                                     start=False, stop=True)
                    nc.tensor.matmul(pnh[:, :w], w_hhT[:, 2 * D:3 * D], h[:, cc],
                                     start=False, stop=True)

                r = gpool.tile([P, CH], F32, tag="r", name="r")
                z = gpool.tile([P, CH], F32, tag="z", name="z")
                nc.scalar.activation(r[:, :w], pr[:, :w], AF.Sigmoid, bias=brc[:])
                nc.scalar.activation(z[:, :w], pz[:, :w], AF.Sigmoid, bias=bzc[:])
                rhn = gpool.tile([P, CH], F32, tag="rhn", name="rhn")
                nc.vector.tensor_mul(rhn[:, :w], r[:, :w], pnh[:, :w])
                t1 = gpool.tile([P, CH], F32, tag="t1", name="t1")
                nc.vector.tensor_add(t1[:, :w], rhn[:, :w], pnx[:, :w])
                nv = gpool.tile([P, CH], F32, tag="nv", name="nv")
                nc.scalar.activation(nv[:, :w], t1[:, :w], AF.Tanh, bias=bihn[:])
                if t == 0:
                    zm = gpool.tile([P, CH], F32, tag="zm", name="zm")
                    nc.vector.tensor_mul(zm[:, :w], z[:, :w], nv[:, :w])
                    nc.vector.tensor_sub(h[:, cc], nv[:, :w], zm[:, :w])
                else:
                    hmn = gpool.tile([P, CH], F32, tag="hmn", name="hmn")
                    nc.vector.tensor_sub(hmn[:, :w], h[:, cc], nv[:, :w])
                    zm = gpool.tile([P, CH], F32, tag="zm", name="zm")
                    nc.vector.tensor_mul(zm[:, :w], z[:, :w], hmn[:, :w])
                    nc.vector.tensor_add(h[:, cc], zm[:, :w], nv[:, :w])

        # ---------------- degree-1 nodes: ft = mean message, no GRU/LN -----
        for c0 in range(0, Nd1, CH):
            w = min(CH, Nd1 - c0)
            off, L = offs[("d1", c0)]
            w32 = L // N_SUB
            S4 = spool.tile([P, L], F32, tag="S4", name="S4d",
                            padded_shape=[P, N_SUB * CH])
            nc.gpsimd.dma_gather(
                out_ap=S4[:].rearrange("p (j e) -> p j e", e=P),
                in_ap=emb, idxs_ap=sb_idx[:, off:off + L // 16],
                num_idxs=L, num_idxs_reg=L, elem_size=P, single_packet=False)
            psumT = trpool.tile([P, CH], F32, tag="tr", name="psumTd")
            for q in range(w32 // 32):
                nc.tensor.matmul(psumT[:, q * 32:(q + 1) * 32],
                                 S4[:, q * P:(q + 1) * P], sbR32[:],
                                 start=True, stop=True)
            nc.scalar.copy(h[:, N2 + c0: N2 + c0 + w32], psumT[:, :w32])

        # ---------------- LayerNorm over D (partitions) for cols [0, N2) --
        for c0 in range(0, N2, CH):
            w = min(CH, N2 - c0)
            cc = slice(c0, c0 + w)
            pmu = ppool.tile([1, CH], F32, tag="pa", name="pmu")
            nc.tensor.matmul(pmu[:1, :w], oneP[:], h[:, cc], start=True, stop=True)
            sq = gpool.tile([P, CH], R32DT, tag="r", name="sq")
            nc.vector.tensor_mul(sq[:, :w], h[:, cc], h[:, cc])
            ps2 = ppool.tile([1, CH], F32, tag="pb", name="ps2")
            nc.tensor.matmul(ps2[:1, :w], oneP[:], sq[:, :w], start=True, stop=True)
            mu = gpool.tile([1, CH], F32, tag="mu", name="mu")
            nc.scalar.copy(mu[:, :w], pmu[:1, :w])
            m2 = gpool.tile([1, CH], F32, tag="m2", name="m2")
            nc.vector.tensor_mul(m2[:, :w], mu[:, :w], mu[:, :w])
            var = gpool.tile([1, CH], F32, tag="var", name="var")
            nc.vector.tensor_sub(var[:, :w], ps2[:1, :w], m2[:, :w])
            std = gpool.tile([1, CH], F32, tag="std", name="std")
            nc.scalar.activation(std[:, :w], var[:, :w], AF.Sqrt, bias=eps_row[:])
            rstd = gpool.tile([1, CH], F32, tag="rstd", name="rstd")
            nc.vector.reciprocal(rstd[:, :w], std[:, :w])
            nmr = gpool.tile([1, CH], F32, tag="nmr", name="nmr")
            nc.vector.tensor_mul(nmr[:, :w], mu[:, :w], rstd[:, :w])
            pa = ppool.tile([P, CH], F32, tag="pc", name="pa_b")
            nc.tensor.matmul(pa[:, :w], ones_f[:1, :P], rstd[:1, :w],
                             start=True, stop=True)
            pb = ppool.tile([P, CH], F32, tag="pd", name="pb_b")
            nc.tensor.matmul(pb[:, :w], ones_f[:1, :P], nmr[:1, :w],
                             start=True, stop=True)
            tl = gpool.tile([P, CH], F32, tag="z", name="tl")
            nc.vector.tensor_mul(tl[:, :w], h[:, cc], pa[:, :w])
            t2 = gpool.tile([P, CH], F32, tag="nv", name="t2")
            nc.vector.tensor_sub(t2[:, :w], tl[:, :w], pb[:, :w])
            nc.vector.tensor_scalar(
                out=h[:, cc], in0=t2[:, :w], scalar1=gamma[:], scalar2=beta[:],
                op0=ALU.mult, op1=ALU.add)

        # ---------------- classifier: outT = wc @ ft + bc ------------------
        for c0 in range(0, Ncol, CH):
            w = min(CH, Ncol - c0)
            cc = slice(c0, c0 + w)
            pcl = ppool.tile([N_CLASSES, CH], F32, tag="pa", name="pcl")
            nc.tensor.matmul(pcl[:N_CLASSES, :w], wcT[:], h[:, cc],
                             start=True, stop=True)
            ot = opool.tile([N_CLASSES, CH], F32, tag="ot", name="ot")
            nc.scalar.activation(ot[:N_CLASSES, :w], pcl[:N_CLASSES, :w],
                                 AF.Identity, bias=bcc[:])
            nc.sync.dma_start(out=outT[:, cc], in_=ot[:N_CLASSES, :w])

    nc.compile()
    return nc


_PROGRAM_CACHE = {}


def _program(sched):
    key = (tuple(sched["n_t"]), sched["Ncol"], sched["kd1"])
    if key not in _PROGRAM_CACHE:
        _PROGRAM_CACHE[key] = _build_program(sched)
    return _PROGRAM_CACHE[key]


# ----------------------------------------------------------------- interface

def _in_maps(inputs):
    sched, idxbufs, colnode = _prep(
        inputs["token_id"], inputs["neigh_idx"], inputs["deg"])
    params = _params(
        inputs["emb"], inputs["w_ih"], inputs["w_hh"], inputs["b_ih"],
        inputs["b_hh"], inputs["gamma"], inputs["beta"], inputs["wc"],
        inputs["bc"])
    maps = [dict(params, idxbuf=idxbufs[c]) for c in range(NCORES)]
    return sched, maps, colnode


def _post(results, sched, colnode):
    out = np.zeros((N_DST, N_CLASSES), np.float32)
    for c in range(NCORES):
        oT = np.asarray(results[c]["outT"])  # [n_classes, Ncol]
        nid = colnode[c]
        valid = nid >= 0
        out[nid[valid]] = oT.T[valid]
    return out


def run(inputs, trace=False):
    sched, maps, colnode = _in_maps(inputs)
    nc = _program(sched)
    res = bass_utils.run_bass_kernel_spmd(
        nc, maps, core_ids=list(range(NCORES)), trace=trace)
    return _post(res.results, sched, colnode), res


def kernel(**inputs) -> np.ndarray:
    out, _ = run(inputs, trace=False)
    return out


# ------------------------------------------------- cached-jit timing harness

class TimedRunner:
    """Build the PJRT executable once; re-invoke on device-resident inputs."""

    def __init__(self, inputs):
        import jax
        from jax.sharding import Mesh, PartitionSpec
        from jax.experimental.shard_map import shard_map
        from concourse import bass2jax

        self.sched, maps, self.colnode = _in_maps(inputs)
        nc = _program(self.sched)
        bass2jax.install_neuronx_cc_hook()

        part_name = (nc.partition_id_tensor.name
                     if nc.partition_id_tensor else None)
        in_names, out_names, out_avals, zero_outs = [], [], [], []
        for alloc in nc.m.functions[0].allocations:
            if not isinstance(alloc, mybir.MemoryLocationSet):
                continue
            name = alloc.memorylocations[0].name
            if alloc.kind == "ExternalInput":
                if name != part_name:
                    in_names.append(name)
            elif alloc.kind == "ExternalOutput":
                out_names.append(name)
                dt_np = mybir.dt.np(alloc.dtype)
                out_avals.append(jax.core.ShapedArray(tuple(alloc.tensor_shape), dt_np))
                zero_outs.append(np.zeros(tuple(alloc.tensor_shape), dt_np))
        n_params = len(in_names)
        all_names = in_names + out_names
        if part_name is not None:
            all_names = all_names + [part_name]

        def _body(*args):
            operands = list(args)
            if part_name is not None:
                operands.append(bass2jax.partition_id_tensor())
            outs = bass2jax._bass_exec_p.bind(
                *operands,
                out_avals=tuple(out_avals),
                in_names=tuple(all_names),
                out_names=tuple(out_names),
                lowering_input_output_aliases=(),
                sim_require_finite=True,
                sim_require_nnan=True,
                nc=nc,
            )
            return tuple(outs)

        devices = jax.devices()[:NCORES]
        mesh = Mesh(np.asarray(devices), ("core",))
        nz = len(zero_outs)
        self.fn = jax.jit(
            shard_map(_body, mesh=mesh,
                      in_specs=(PartitionSpec("core"),) * (n_params + nz),
                      out_specs=(PartitionSpec("core"),) * nz,
                      check_rep=False),
            keep_unused=True)
        concat_in = [np.concatenate([np.asarray(m[n]) for m in maps], axis=0)
                     for n in in_names]
        self.dev_in = [jax.device_put(a) for a in concat_in]
        self.zero_outs = zero_outs
        self.out_names = out_names
        self.out_avals = out_avals
        self.jax = jax

    def __call__(self):
        zeros = [np.zeros((NCORES * z.shape[0], *z.shape[1:]), z.dtype)
                 for z in self.zero_outs]
        outs = self.fn(*self.dev_in, *zeros)
        self.jax.block_until_ready(outs)
        return outs

    def timed(self, iters=5):
        import time
        self()  # warm-up / compile
        times = []
        for _ in range(iters):
            t0 = time.perf_counter()
            self()
            times.append(time.perf_counter() - t0)
        return min(times)

    def result(self):
        outs = self()
        results = []
        for c in range(NCORES):
            d = {}
            for i, n in enumerate(self.out_names):
                full = np.asarray(outs[i])
                d[n] = full.reshape(NCORES, *self.out_avals[i].shape)[c]
            results.append(d)
        return _post(results, self.sched, self.colnode)

